# revision 26
# baseline (speedup 1.0000x reference)
"""MoE-with-DeepGEMM kernel for 8 Trainium2 NeuronCores.

Problem: M=4096 tokens, D=2048 in-dim, H=2048 out-dim, E=8 experts.
    gate = softmax(x @ gate_w.T + gate_b)            # [M, E], fp32
    y    = (q8(x) @ q8(expert_w[e]).T) -> bf16       # [E, M, H]
    out  = sum_e gate[:, e, None] * y[e].astype(f32) # [M, H]

Strategy: data-parallel over tokens (M). Each of the 8 cores gets
M/8 = 512 tokens, all 8 experts' weights, and computes its output slice
independently — no collectives; the host concatenates the slices.

Differences vs the first working version (252.4 us):
  - All DRAM tensors are host-prepermuted so every DMA is contiguous
    per partition (128 descriptors of 4-32 KB instead of ~25k of
    0.5-2 KB). Cuts descriptor-generation latency at startup and DMA
    engine occupancy throughout.
  - PE warmup: a short chain of dummy matmuls on a zeroed scratch tile
    runs while the first weights stream in, so the HAM clock-gate is
    released (2.4 GHz) before the first real matmul.
  - The per-expert combine for e1..e7 reads PSUM f32 directly with a
    DVE scalar_tensor_tensor (skipping the reference's bf16 cast of y;
    +0.1% uncorrelated error, well inside the 2e-2 budget). Only e0
    still drains through fast ACT bf16 copies, because its PSUM banks
    must be freed quickly while w0 is still streaming.
  - Softmax without PE transposes: logits are small (|l| <~ 6) so no
    max-subtraction is needed; the [E, M] -> [M, E] layout flip is done
    with 16 DVE 32x32 block transposes.
  - Tail: expert 7 writes gate*psum + acc as bf16 tiles that stream to
    HBM per 512-column chunk on the Activation HWDGE ring; the host
    upcasts to f32. The last m-chunk runs its k-loop in two hc-halves
    so the final drain is one STT + one 128 KB DMA.
"""

import numpy as np
import ml_dtypes

import concourse.bacc as bacc
import concourse.bass as bass
import concourse.mybir as mybir
import concourse.tile as tile
from concourse.tile import add_dep_helper
from concourse.bass_utils import run_bass_kernel_spmd

M, D, H, E = 4096, 2048, 2048, 8
NCORES = 8
MS = M // NCORES          # tokens per core (512)
MC = MS // 128            # m-chunks of 128 partitions (4)
DS = D // 128             # d-subtiles of 128 (16)
KP = DS // 2              # DoubleRow d-pairs of 256 (8)
NH = 512                  # h columns per matmul (one PSUM bank of f32)
HC = H // NH              # h-chunks (4)
NWARM = 13                # dummy warmup matmuls (N=256 fp8)

_NC = None


def _build_program() -> bass.Bass:
    dt = mybir.dt
    nc = bacc.Bacc(None, target_bir_lowering=False)

    # Host-prepermuted layouts. Row index encodes (p, s) with d = s*128+p,
    # so a row-range rearrange("(p s) x -> p s x") gives per-partition
    # contiguous descriptors for any s-slice.
    xq = nc.dram_tensor("xq", [128 * DS, MS], dt.float8e4, kind="ExternalInput")
    xf = nc.dram_tensor("xf", [128 * DS, MS], dt.float16, kind="ExternalInput")
    wq = nc.dram_tensor("wq", [E * 128 * DS, H], dt.float8e4, kind="ExternalInput")
    gwt = nc.dram_tensor("gwt", [128 * DS, 32], dt.float16, kind="ExternalInput")
    gb = nc.dram_tensor("gb", [E, 1], dt.float32, kind="ExternalInput")
    out = nc.dram_tensor("out", [MS, H], dt.bfloat16, kind="ExternalOutput")

    with tile.TileContext(nc) as tc, \
            tc.tile_pool(name="const", bufs=1) as constp, \
            tc.tile_pool(name="wpool", bufs=2) as wpool, \
            tc.tile_pool(name="ypool", bufs=16) as ypool, \
            tc.tile_pool(name="opool", bufs=4) as opool, \
            tc.tile_pool(name="small", bufs=8) as small, \
            tc.tile_pool(name="ps", bufs=8, space="PSUM") as psp:

        xq_sb = constp.tile([128, DS, MS], dt.float8e4, tag="xq")
        xf_sb = constp.tile([128, DS, MS], dt.float16, tag="xf")
        gwt_sb = constp.tile([128, DS, 32], dt.float16, tag="gwt")
        gb_sb = constp.tile([E, 1], dt.float32, tag="gb")
        lg_sb = constp.tile([32, MS], dt.float32, tag="lg")
        gt_sb = constp.tile([128, MC * 32], dt.float32, tag="gt")
        gate_sb = constp.tile([128, MC * E], dt.float32, tag="gate")
        acc_sb = constp.tile([128, MC * H], dt.float32, tag="acc")
        zscr = constp.tile([128, 640], dt.float8e4, tag="zscr")

        # ---- startup DMAs ----
        # Two HWDGE rings run concurrently: the Sync ring carries the
        # k-ordered w chunks (+xf after w0); the Scalar ring carries
        # xq/gwt/gb and later the output tiles. No explicit DMA-DMA deps:
        # a dep is a completion-semaphore wait (~2 us receipt) — ring
        # FIFO order gives the sequencing for free.
        w_tiles = {}
        firsts = {}

        def emit_w_dmas(e, nchunks, split_first=False):
            w_sb = wpool.tile([128, DS, H], dt.float8e4, tag="w")
            w_tiles[e] = w_sb
            eap = wq[e * 128 * DS:(e + 1) * 128 * DS, :].rearrange(
                "(p s) h -> p s h", p=128)
            sch = DS // nchunks
            last = None
            firsts[e] = None
            for c in range(nchunks):
                if c == 0 and split_first:
                    # h-halves of the first k-pair: the very first matmuls
                    # (k0, hc0-1) wait on only 0.26 MB
                    for hh in range(2):
                        last = nc.sync.dma_start(
                            w_sb[:, 0:sch, hh * 1024:(hh + 1) * 1024],
                            eap[:, 0:sch, hh * 1024:(hh + 1) * 1024],
                        )
                    continue
                last = nc.sync.dma_start(
                    w_sb[:, c * sch:(c + 1) * sch, :],
                    eap[:, c * sch:(c + 1) * sch, :],
                )
                if firsts[e] is None:
                    firsts[e] = last
            return last

        # expert 0 in 8 k-pair chunks so compute can chase arrival.
        w0_last = emit_w_dmas(0, 8)
        # xq: first k-group slice, then the rest (the first matmuls wait
        # only on the small slice + the first w chunk).
        xq_ap = xq[:, :].rearrange("(p s) m -> p s m", p=128)
        d_xqa = nc.scalar.dma_start(xq_sb[:, 0:2, :], xq_ap[:, 0:2, :])
        # gwt/gb next: their ~2us of serialized descriptor-gen delays the
        # xq remainder, so the w0 chunks expert 0 is chasing get nearly
        # full DMA bandwidth during the critical first ~2us of flow.
        nc.scalar.dma_start(gwt_sb[:], gwt[:, :].rearrange("(p s) e -> p s e", p=128))
        nc.scalar.dma_start(gb_sb[:], gb[:, :])
        nc.scalar.dma_start(xq_sb[:, 2:4, :], xq_ap[:, 2:4, :])
        nc.scalar.dma_start(xq_sb[:, 4:8, :], xq_ap[:, 4:8, :])
        nc.scalar.dma_start(xq_sb[:, 8:DS, :], xq_ap[:, 8:DS, :])
        # gating inputs stream only after w0 is fully resident (explicit
        # dep: concurrent DMAs fair-share bandwidth, and xf would dilute
        # the w0 chunks expert 0 is chasing). Needed ~15 us later.
        d_xf = nc.sync.dma_start(
            xf_sb[:], xf[:, :].rearrange("(p s) m -> p s m", p=128))
        add_dep_helper(d_xf.ins, w0_last.ins, reason="xf after w0")

        # ---- PE warmup: dummy matmuls on zeroed scratch while w0 lands ----
        nc.vector.memset(zscr[:], 0)
        ps_warm = psp.tile([128, NH], dt.float32, tag="ps", name="ps_warm")
        last_warm = None
        for i in range(NWARM):
            last_warm = nc.tensor.matmul(
                ps_warm[:, 0:256], lhsT=zscr[:, 0:128], rhs=zscr[:, 128:128 + 256],
                start=True, stop=True,
            )
        for i in range(6):
            last_warm = nc.tensor.matmul(
                ps_warm[:, 0:128], lhsT=zscr[:, 0:128], rhs=zscr[:, 128:256],
                start=True, stop=True,
            )

        # ---- gating (emitted at the e0/e1 boundary) ----
        # Column-tiled: 4 independent 128x32 PE tiles compute 4 d-subtiles
        # concurrently (the gate output is only 8 rows, padded to 32, so a
        # full-array matmul wastes 15/16 of the PE). 4 passes x 4 tiles
        # cover all 16 d-subtiles in ~1/4 the PE time; partials land on
        # PSUM partition groups {0,32,64,96}, combined by a DVE add chain
        # (DVE may read only ONE input from PSUM per instruction).
        def emit_gating():
            ps_gt = psp.tile([128, MS], dt.float32, tag="ps", name="ps_gt")
            for p in range(4):
                for ci in range(4):
                    s = p * 4 + ci
                    nc.tensor.matmul(
                        ps_gt[32 * ci:32 * (ci + 1), :],
                        lhsT=gwt_sb[:, s:s + 1, :],
                        rhs=xf_sb[:, s:s + 1, :],
                        start=(p == 0),
                        stop=(p == 3),
                        tile_position=(0, 32 * ci),
                        skip_group_check=True,
                    )
            t0 = small.tile([E, MS], dt.float32, tag="gs")
            nc.vector.tensor_scalar_add(t0[:], ps_gt[0:E, :], gb_sb[:])
            nc.vector.tensor_tensor(
                t0[:], t0[:], ps_gt[32:32 + E, :], op=mybir.AluOpType.add)
            nc.vector.tensor_tensor(
                t0[:], t0[:], ps_gt[64:64 + E, :], op=mybir.AluOpType.add)
            nc.gpsimd.memset(lg_sb[:], 0)
            nc.vector.tensor_tensor(
                lg_sb[0:E, :], t0[:], ps_gt[96:96 + E, :], op=mybir.AluOpType.add)
            # [E, M] -> [M, E] via 32x32 DVE block transposes, then an
            # unstabilized softmax (logits are O(1); f32 exp is safe).
            for mc in range(MC):
                for b in range(4):
                    nc.vector.transpose(
                        gt_sb[32 * b:32 * (b + 1), mc * 32:(mc + 1) * 32],
                        lg_sb[:, mc * 128 + 32 * b:mc * 128 + 32 * (b + 1)],
                    )
            for mc in range(MC):
                ex = small.tile([128, E], dt.float32, tag="sm")
                ssum = small.tile([128, 1], dt.float32, tag="sm1")
                nc.scalar.activation(
                    ex[:], gt_sb[:, mc * 32:mc * 32 + E],
                    mybir.ActivationFunctionType.Exp,
                    bias=0.0, scale=1.0, accum_out=ssum[:],
                )
                rcp = small.tile([128, 1], dt.float32, tag="sm1")
                nc.vector.reciprocal(rcp[:], ssum[:])
                nc.vector.tensor_scalar_mul(
                    gate_sb[:, mc * E:(mc + 1) * E], ex[:], rcp[:]
                )

        # ---- expert 0: two k-major mc-pair phases; fast ACT bf16 drains ----
        w0 = w_tiles[0]
        # absorb the first data-wait into a PE nop so the first real
        # matmul's LDWEIGHTS can prefetch during the last warmup matmul
        t_nop = nc.tensor.nop()
        add_dep_helper(t_nop.ins, last_warm.ins, reason="order after warmup")
        add_dep_helper(t_nop.ins, firsts[0].ins, reason="absorb w0c0 wait")
        add_dep_helper(t_nop.ins, d_xqa.ins, reason="absorb xqA wait")
        last_warm = t_nop
        y_hold = []
        first_mm = True
        for phase_mcs in ((0, 1), (2, 3)):
            pss = {
                mc: [psp.tile([128, NH], dt.float32, tag="ps",
                              name=f"ps0_{mc}_{i}") for i in range(HC)]
                for mc in phase_mcs
            }
            for k in range(KP):
                for mc in phase_mcs:
                    lhsT = xq_sb[:, 2 * k:2 * k + 2, mc * 128:(mc + 1) * 128]
                    for hc in range(HC):
                        mm = nc.tensor.matmul(
                            pss[mc][hc][:],
                            lhsT=lhsT,
                            rhs=w0[:, 2 * k:2 * k + 2, hc * NH:(hc + 1) * NH],
                            start=(k == 0),
                            stop=(k == KP - 1),
                            perf_mode=mybir.MatmulPerfMode.DoubleRow,
                        )
                        if first_mm:
                            add_dep_helper(mm.ins, last_warm.ins,
                                           reason="after warmup")
                            first_mm = False
            for mc in phase_mcs:
                for hc in range(HC):
                    y = ypool.tile([128, NH], dt.bfloat16, tag="y")
                    nc.scalar.copy(y[:], pss[mc][hc][:])
                    y_hold.append((mc, hc, y))

        emit_w_dmas(1, 2)
        emit_gating()
        for mc0, hc0, y0 in y_hold:
            nc.vector.tensor_scalar_mul(
                acc_sb[:, mc0 * H + hc0 * NH:mc0 * H + (hc0 + 1) * NH],
                y0[:], gate_sb[:, mc0 * E:mc0 * E + 1],
            )

        # ---- experts 1..7: direct-PSUM DVE combine ----
        for e in range(1, E):
            if e >= 2:
                emit_w_dmas(e, 1)
            w_sb = w_tiles[e]
            for mc in range(MC):
                msl = slice(mc * 128, (mc + 1) * 128)
                g_ap = gate_sb[:, mc * E + e:mc * E + e + 1]

                def do_kloop(hcs, pss):
                    for k in range(KP):
                        lhsT = xq_sb[:, 2 * k:2 * k + 2, msl]
                        for hc in hcs:
                            nc.tensor.matmul(
                                pss[hc][:],
                                lhsT=lhsT,
                                rhs=w_sb[:, 2 * k:2 * k + 2,
                                         hc * NH:(hc + 1) * NH],
                                start=(k == 0),
                                stop=(k == KP - 1),
                                perf_mode=mybir.MatmulPerfMode.DoubleRow,
                            )

                def drain(hcs, pss):
                    for hc in hcs:
                        a_ap = acc_sb[:, mc * H + hc * NH:mc * H + (hc + 1) * NH]
                        if e == E - 1:
                            ob = opool.tile([128, NH], dt.bfloat16, tag="ob")
                            nc.vector.scalar_tensor_tensor(
                                ob[:], pss[hc][:], g_ap, a_ap,
                                op0=mybir.AluOpType.mult,
                                op1=mybir.AluOpType.add,
                            )
                            eng = nc.scalar if hc % 2 == 0 else nc.sync
                            eng.dma_start(
                                out[msl, hc * NH:(hc + 1) * NH], ob[:])
                        else:
                            nc.vector.scalar_tensor_tensor(
                                a_ap, pss[hc][:], g_ap, a_ap,
                                op0=mybir.AluOpType.mult,
                                op1=mybir.AluOpType.add,
                            )

                if e == E - 1 and mc == MC - 1:
                    # last m-chunk: (hc0,hc1), then hc2, then hc3, so the
                    # final drain is one STT + one 128 KB DMA.
                    for hcs in ((0, 1), (2,), (3,)):
                        pss = {hc: psp.tile([128, NH], dt.float32, tag="ps",
                                            name=f"ps_{e}_{mc}_{hc}")
                               for hc in hcs}
                        do_kloop(hcs, pss)
                        drain(hcs, pss)
                else:
                    pss = {hc: psp.tile([128, NH], dt.float32, tag="ps",
                                        name=f"ps_{e}_{mc}_{hc}")
                           for hc in range(HC)}
                    do_kloop(range(HC), pss)
                    drain(range(HC), pss)

    nc.compile()
    return nc


def _get_nc() -> bass.Bass:
    global _NC
    if _NC is None:
        _NC = _build_program()
    return _NC


def _perm_ps(a2d, np_dtype):
    """[D, N] -> [(p s), N] rows with d = s*128 + p, contiguous."""
    d, n = a2d.shape
    s = d // 128
    return np.ascontiguousarray(
        a2d.reshape(s, 128, n).transpose(1, 0, 2).astype(np_dtype)
    ).reshape(d, n)


def _prep_in_maps(x, gate_w, gate_b, expert_w):
    f8fn = ml_dtypes.float8_e4m3fn
    f8trn = ml_dtypes.float8_e4m3  # same bits as e4m3fn for |v| <= 240

    x = np.asarray(x, dtype=np.float32)
    gate_w = np.asarray(gate_w, dtype=np.float32)
    gate_b = np.asarray(gate_b, dtype=np.float32)
    expert_w = np.asarray(expert_w, dtype=np.float32)

    xT = np.ascontiguousarray(x.T)                       # [D, M] f32
    xqT = xT.astype(f8fn).view(f8trn)                    # [D, M] fp8
    xfT = xT.astype(np.float16)                          # [D, M] fp16
    # expert_w [E, H, D] -> per expert [D, H] -> (p s)-permuted rows.
    wqT = expert_w.transpose(0, 2, 1).astype(f8fn).view(f8trn)  # [E, D, H]
    wq_rows = np.ascontiguousarray(
        wqT.reshape(E, DS, 128, H).transpose(0, 2, 1, 3)
    ).reshape(E * D, H)
    gwt_pad = np.zeros((D, 32), dtype=np.float32)
    gwt_pad[:, :E] = gate_w.T
    gwt = _perm_ps(gwt_pad, np.float16)  # [D, 32] (zero-padded experts)
    gbb = np.ascontiguousarray(gate_b.reshape(E, 1))

    in_maps = []
    for c in range(NCORES):
        csl = slice(c * MS, (c + 1) * MS)
        in_maps.append({
            "xq": _perm_ps(xqT[:, csl], f8fn).view(f8trn),
            "xf": _perm_ps(xfT[:, csl], np.float16),
            "wq": wq_rows,
            "gwt": gwt,
            "gb": gbb,
        })
    return in_maps


def kernel(x, gate_w, gate_b, expert_w, _trace=False, _trace_kwargs=None):
    nc = _get_nc()
    in_maps = _prep_in_maps(x, gate_w, gate_b, expert_w)
    kw = {}
    if _trace:
        kw["trace"] = True
        kw.update(_trace_kwargs or {})
    res = run_bass_kernel_spmd(nc, in_maps, core_ids=list(range(NCORES)), **kw)
    outp = np.concatenate(
        [np.asarray(res.results[c]["out"]) for c in range(NCORES)], axis=0
    ).astype(np.float32)
    if _trace:
        return outp, res
    return outp
